# revision 6
# baseline (speedup 1.0000x reference)
"""DenseToSparse forward on 8 Trainium2 NeuronCores.

Input : x [32, 64, 128, 128] f32 (NCHW; ~50% of spatial sites are all-zero
        across channels).
Output: (feats [32*128*128, 64] f32, count int32) — NHWC rows at active sites
        compacted to the front in row-major site order, zero padding after.

Strategy (data parallel, 4 samples per core):
  * Per 128-site tile: one PE matmul  x_tile[64,128]^T @ [I64 | 1]  ->
    PSUM [128, 65] = transposed rows plus a channel-sum column (site activity).
  * Prefix sums: DVE scan within tiles + strict-upper-triangular ones matmul
    across tiles -> per-site compacted destination row.
  * One indirect DMA per sample scatters the 16384 rows (256B each) to their
    destination rows; inactive sites get a huge destination and are skipped
    via the DMA bounds check, so only active rows are written.
  * Host concatenates the per-sample active prefixes (counts come back as a
    small per-tile-totals tensor).
"""

import numpy as np

_B, _C, _H, _W = 32, 64, 128, 128
_NCORES = 8
_SPC = _B // _NCORES          # samples per core
_S = _H * _W                  # sites per sample
_BIG = float(2 << 23)         # inactive-row destination offset (OOB -> skipped)


def _build_tile(tc, x_ap, feats_ap, counts_ap, n_samples, n_chan, n_sites):
    """Emit the tile program. x [n_samples, n_chan, n_sites] f32,
    feats [n_samples*n_sites, n_chan] f32, counts [n_samples, n_sites//128] f32."""
    from contextlib import ExitStack

    import concourse.bass as bass
    from concourse import mybir

    nc = tc.nc
    P = 128
    tiles = n_sites // P          # site tiles per sample
    half = n_sites // 2           # sites per partition-half buffer
    half_tiles = tiles // 2
    assert tiles % 4 == 0 and n_chan == 64 and n_sites % 256 == 0

    f32 = mybir.dt.float32
    i32 = mybir.dt.int32

    # ---- constants (baked into the NEFF) ----
    aug_np = np.zeros((P, n_chan + 1), np.float32)
    aug_np[:n_chan, :n_chan] = np.eye(n_chan, dtype=np.float32)
    aug_np[:n_chan, n_chan] = 1.0
    aug_np[64:64 + n_chan] = aug_np[:n_chan]          # copy for partition-half B
    id_np = np.eye(P, dtype=np.float32)
    ut_np = np.triu(np.ones((P, P), np.float32), 1)   # UT[k,m]=1 iff k<m

    aug_dram = nc.inline_tensor(aug_np, name="caug")
    id_dram = nc.inline_tensor(id_np, name="cid")
    ut_dram = nc.inline_tensor(ut_np, name="cut")

    with ExitStack() as ctx:
        consts = ctx.enter_context(tc.tile_pool(name="consts", bufs=1))
        xpool = ctx.enter_context(tc.tile_pool(name="xpool", bufs=2))
        aapool = ctx.enter_context(tc.tile_pool(name="aapool", bufs=2))
        vpool = ctx.enter_context(tc.tile_pool(name="vpool", bufs=2))
        opool = ctx.enter_context(tc.tile_pool(name="opool", bufs=2))
        pspool = ctx.enter_context(tc.tile_pool(name="ps", bufs=4, space="PSUM"))
        smps = ctx.enter_context(tc.tile_pool(name="smps", bufs=1, space="PSUM"))

        aug_sb = consts.tile([P, n_chan + 1], f32)
        id_sb = consts.tile([P, P], f32)
        ut_sb = consts.tile([P, P], f32)
        zeros_sb = consts.tile([P, P], f32)
        nc.sync.dma_start(out=aug_sb[:], in_=aug_dram[:])
        nc.sync.dma_start(out=id_sb[:], in_=id_dram[:])
        nc.sync.dma_start(out=ut_sb[:], in_=ut_dram[:])
        nc.vector.memset(zeros_sb[:], 0.0)

        for b in range(n_samples):
            # ---- load sample: two 64-partition halves -> full 128 partitions
            x_sb = xpool.tile([P, half], f32)
            nc.sync.dma_start(out=x_sb[0:64, :], in_=x_ap[b, :, 0:half])
            nc.sync.dma_start(out=x_sb[64:128, :], in_=x_ap[b, :, half:n_sites])

            # ---- transpose + site sums: per tile  out[128,65] = x_t^T @ [I|1]
            aa_sb = aapool.tile([P, tiles * n_chan], f32)
            aa3 = aa_sb[:].rearrange("p (t c) -> p t c", c=n_chan)
            mask_pt = vpool.tile([P, tiles], f32, tag="mask_pt")
            for g in range(tiles // 4):
                ps = pspool.tile([P, 4 * (n_chan + 1)], f32, space="PSUM")
                ps3 = ps[:].rearrange("p (i c) -> p i c", c=n_chan + 1)
                for i in range(4):
                    t = 4 * g + i
                    hb = 0 if t < half_tiles else 64
                    col = P * (t % half_tiles)
                    nc.tensor.matmul(
                        out=ps[:, i * (n_chan + 1):(i + 1) * (n_chan + 1)],
                        lhsT=x_sb[hb:hb + 64, col:col + P],
                        rhs=aug_sb[hb:hb + 64, :],
                        start=True,
                        stop=True,
                    )
                # rows -> aa_sb (alternate engines), sums column -> mask_pt
                if g % 2 == 0:
                    nc.vector.tensor_copy(aa3[:, 4 * g:4 * g + 4, :], ps3[:, :, 0:n_chan])
                else:
                    nc.scalar.copy(aa3[:, 4 * g:4 * g + 4, :], ps3[:, :, 0:n_chan])
                nc.vector.tensor_copy(mask_pt[:, 4 * g:4 * g + 4], ps3[:, :, n_chan])

            # ---- destinations: transpose mask to [tile, site-in-tile] layout
            mask_tp = smps.tile([tiles, P], f32, space="PSUM", tag="mask_tp")
            nc.tensor.transpose(out=mask_tp[:], in_=mask_pt[:], identity=id_sb[:])

            active = vpool.tile([tiles, P], f32, tag="active")
            nc.vector.tensor_scalar(
                out=active[:], in0=mask_tp[:], scalar1=0.0, scalar2=None,
                op0=mybir.AluOpType.not_equal,
            )
            incl = vpool.tile([tiles, P], f32, tag="incl")
            nc.vector.tensor_tensor_scan(
                out=incl[:], data0=active[:], data1=zeros_sb[0:tiles, :],
                initial=0.0, op0=mybir.AluOpType.add, op1=mybir.AluOpType.add,
            )
            # within-tile exclusive prefix
            exw = vpool.tile([tiles, P], f32, tag="exw")
            nc.vector.tensor_tensor(
                out=exw[:], in0=incl[:], in1=active[:], op=mybir.AluOpType.subtract
            )
            # cross-tile exclusive prefix of per-tile totals
            exclt = smps.tile([tiles, 1], f32, space="PSUM", tag="exclt")
            nc.tensor.matmul(
                out=exclt[:], lhsT=ut_sb[0:tiles, 0:tiles],
                rhs=incl[:, P - 1:P], start=True, stop=True,
            )
            # dest = exw + exclt + sample_base  (active) | BIG (inactive)
            dest = vpool.tile([tiles, P], f32, tag="dest")
            nc.vector.tensor_scalar(
                out=dest[:], in0=exw[:], scalar1=exclt[:, 0:1],
                scalar2=float(b * n_sites),
                op0=mybir.AluOpType.add, op1=mybir.AluOpType.add,
            )
            # dfin = dest + BIG*(1-active): exact for active (adds 0.0), OOB for
            # inactive (walrus rejects f32 masks on CopyPredicated, so no select)
            m1 = vpool.tile([tiles, P], f32, tag="m1")
            nc.vector.tensor_scalar(
                out=m1[:], in0=active[:], scalar1=-_BIG, scalar2=_BIG,
                op0=mybir.AluOpType.mult, op1=mybir.AluOpType.add,
            )
            dfin = vpool.tile([tiles, P], f32, tag="dfin")
            nc.vector.tensor_tensor(
                out=dfin[:], in0=dest[:], in1=m1[:], op=mybir.AluOpType.add
            )
            # back to [site-in-tile, tile] layout, cast to int32
            dest_ps = smps.tile([P, tiles], f32, space="PSUM", tag="dest_ps")
            nc.tensor.transpose(
                out=dest_ps[:], in_=dfin[:], identity=id_sb[0:tiles, 0:tiles]
            )
            offs = opool.tile([P, tiles], i32)
            nc.vector.tensor_copy(offs[:], dest_ps[:])

            # ---- scatter rows tile by tile; inactive rows are OOB and skipped.
            # HW DGE pairs one offset per partition ([P,1]) with that
            # partition's row — multi-column offset APs misbehave on silicon.
            for t in range(tiles):
                nc.gpsimd.indirect_dma_start(
                    out=feats_ap,
                    out_offset=bass.IndirectOffsetOnAxis(
                        ap=offs[:, t:t + 1], axis=0),
                    in_=aa3[:, t, :],
                    in_offset=None,
                    bounds_check=n_samples * n_sites - 1,
                    oob_is_err=False,
                )

            # ---- per-tile totals -> host computes counts
            nc.sync.dma_start(out=counts_ap[b, 0:tiles], in_=incl[:, P - 1:P])


def build_module(n_samples=_SPC, n_chan=_C, n_sites=_S):
    """Build + compile the per-core Bass module."""
    from concourse import bacc, mybir
    import concourse.tile as tile

    nc = bacc.Bacc("TRN2")
    x = nc.dram_tensor("x", [n_samples, n_chan, n_sites], mybir.dt.float32,
                       kind="ExternalInput")
    feats = nc.dram_tensor("feats", [n_samples * n_sites, n_chan],
                           mybir.dt.float32, kind="ExternalOutput")
    counts = nc.dram_tensor("counts", [n_samples, n_sites // 128],
                            mybir.dt.float32, kind="ExternalOutput")
    with tile.TileContext(nc) as tc:
        _build_tile(tc, x[:], feats[:], counts[:], n_samples, n_chan, n_sites)
    nc.compile()
    return nc


_CACHED = {}


def _get_module():
    if "nc" not in _CACHED:
        _CACHED["nc"] = build_module()
    return _CACHED["nc"]


def kernel(x: np.ndarray):
    """Full DenseToSparse forward. x [32, 64, 128, 128] f32 ->
    (feats [32*128*128, 64] f32, count int32)."""
    from concourse.bass_utils import run_bass_kernel_spmd

    assert x.shape == (_B, _C, _H, _W), x.shape
    x = np.ascontiguousarray(x, dtype=np.float32)
    xs = x.reshape(_B, _C, _S)

    nc = _get_module()
    in_maps = [
        {"x": np.ascontiguousarray(xs[k * _SPC:(k + 1) * _SPC])}
        for k in range(_NCORES)
    ]
    res = run_bass_kernel_spmd(nc, in_maps, core_ids=list(range(_NCORES)))
    global _LAST_RESULTS
    _LAST_RESULTS = res

    feats = np.zeros((_B * _S, _C), np.float32)
    off = 0
    for k in range(_NCORES):
        fk = res.results[k]["feats"]
        ck = res.results[k]["counts"]
        for j in range(_SPC):
            cnt = int(round(float(ck[j].sum())))
            if cnt:
                feats[off:off + cnt] = fk[j * _S:j * _S + cnt]
            off += cnt
    return feats, np.int32(off)
